# revision 9
# baseline (speedup 1.0000x reference)
"""Trainium2 Bass kernel for nn_CausalDiscoveryLayer (27-node Granger + MHA).

Contract: kernel(**inputs) takes FULL unsharded numpy inputs and returns the
full output (out [27,1024,5], causal_adj [27,27]) matching the reference.

Strategy (8 NeuronCores, tensor-parallel on the two big weight matrices):
  Launch 1: in_proj (15360x5120, 315MB) sharded 1920 rows/core; each core
            computes its qkv slice [27,1920]; Granger adjacency replicated.
  Host:     reassemble qkv, carve per-head q^T,k^T and v slices.
  Launch 2: out_proj (5120x5120, 105MB) sharded over the contraction dim
            (640/core = half a head); each core runs softmax-attention for
            its head and produces a partial output [27,5120]; host sums.

Weights are transposed on the host so that the contraction dimension lies on
SBUF partitions with unit-stride DMA (fp32 has no HW DMA-transpose path).
"""

import numpy as np

import concourse.bass as bass
import concourse.bacc as bacc
import concourse.mybir as mybir
import concourse.tile as tile
from concourse.bass_utils import run_bass_kernel_spmd
from concourse.masks import make_identity

N = 27
D = 1024
E = 5120
H = 4
HD = E // H          # 1280
NCORES = 8
P = 128
KC = E // P          # 40 contraction chunks of 128
JC = 3 * E // NCORES  # 1920 in_proj output cols per core
JT = 480             # matmul free-dim tile for launch 1 (4 * 480 = 1920)
FS = E // NCORES     # 640 out_proj contraction rows per core
FP32 = mybir.dt.float32

# Results of the last run (BassKernelResults per launch) for test harnesses.
LAST_RESULTS = []

_NC_CACHE = {}


def _build_launch1():
    nc = bacc.Bacc("TRN2", target_bir_lowering=False, debug=False,
                   num_devices=NCORES)
    wt = nc.dram_tensor("wt", [E, JC], FP32, kind="ExternalInput")
    xT = nc.dram_tensor("xT", [P, KC, N], FP32, kind="ExternalInput")
    histT = nc.dram_tensor("histT", [P, KC, N], FP32, kind="ExternalInput")
    wn = nc.dram_tensor("wn", [P, KC, 1], FP32, kind="ExternalInput")
    wh = nc.dram_tensor("wh", [P, KC, 1], FP32, kind="ExternalInput")
    gb = nc.dram_tensor("gb", [1, 1], FP32, kind="ExternalInput")
    bias = nc.dram_tensor("bias", [1, JC], FP32, kind="ExternalInput")
    mask = nc.dram_tensor("mask", [N, N], FP32, kind="ExternalInput")
    qkv = nc.dram_tensor("qkv", [N, JC], FP32, kind="ExternalOutput")
    adj = nc.dram_tensor("adj", [N, N], FP32, kind="ExternalOutput")

    with tile.TileContext(nc) as tc:
        with (
            tc.tile_pool(name="const", bufs=1) as const,
            tc.tile_pool(name="rhs", bufs=4) as rhsp,
            tc.tile_pool(name="outsb", bufs=1) as outsb,
            tc.tile_pool(name="acc", bufs=4, space="PSUM") as accp,
            tc.tile_pool(name="gps", bufs=1, space="PSUM") as gpsp,
        ):
            ones = const.tile([1, N], FP32)
            nc.gpsimd.memset(ones[:], 1.0)
            xT_sb = const.tile([P, KC, N], FP32)
            nc.sync.dma_start(xT_sb[:], xT[:])
            histT_sb = const.tile([P, KC, N], FP32)
            nc.sync.dma_start(histT_sb[:], histT[:])
            wn_sb = const.tile([P, KC, 1], FP32)
            nc.sync.dma_start(wn_sb[:], wn[:])
            wh_sb = const.tile([P, KC, 1], FP32)
            nc.sync.dma_start(wh_sb[:], wh[:])
            gb_sb = const.tile([1, 1], FP32)
            nc.sync.dma_start(gb_sb[:], gb[:])
            bias_sb = const.tile([1, JC], FP32)
            nc.sync.dma_start(bias_sb[:], bias[:])
            mask_sb = const.tile([N, N], FP32)
            nc.sync.dma_start(mask_sb[:], mask[:])

            out_sb = outsb.tile([N, JC], FP32)

            # Main projection: qkv_slice[n, j] = sum_k x[n,k] * W^T[k, j] + b[j]
            psums = [accp.tile([N, JT], FP32, tag="acc", name=f"acc{j}")
                     for j in range(JC // JT)]
            for kc in range(KC):
                rhs = rhsp.tile([P, JC], FP32)
                nc.sync.dma_start(rhs[:], wt[kc * P:(kc + 1) * P, :])
                for jc in range(JC // JT):
                    nc.tensor.matmul(
                        psums[jc][:],
                        xT_sb[:, kc, :],
                        rhs[:, jc * JT:(jc + 1) * JT],
                        start=(kc == 0), stop=False,
                    )
            for jc in range(JC // JT):
                # bias broadcast: ones^T [N,1] @ bias_chunk [1,JT]
                nc.tensor.matmul(
                    psums[jc][:], ones[:],
                    bias_sb[:, jc * JT:(jc + 1) * JT],
                    start=False, stop=True,
                )
                nc.vector.tensor_copy(out_sb[:, jc * JT:(jc + 1) * JT], psums[jc][:])
            nc.sync.dma_start(qkv[:], out_sb[:])

            # Granger: col[i] = x[i,:].wn + gb ; row[j] = hist[j,:].wh
            col_ps = gpsp.tile([N, 1], FP32, tag="col")
            for kc in range(KC):
                nc.tensor.matmul(col_ps[:], xT_sb[:, kc, :], wn_sb[:, kc, :],
                                 start=(kc == 0), stop=False)
            nc.tensor.matmul(col_ps[:], ones[:], gb_sb[:], start=False, stop=True)
            row_ps = gpsp.tile([1, N], FP32, tag="row")
            for kc in range(KC):
                nc.tensor.matmul(row_ps[:], wh_sb[:, kc, :], histT_sb[:, kc, :],
                                 start=(kc == 0), stop=(kc == KC - 1))
            col_sb = const.tile([N, 1], FP32)
            nc.vector.tensor_copy(col_sb[:], col_ps[:])
            row_sb = const.tile([1, N], FP32)
            nc.vector.tensor_copy(row_sb[:], row_ps[:])
            rowb_ps = gpsp.tile([N, N], FP32, tag="rowb")
            nc.tensor.matmul(rowb_ps[:], ones[:], row_sb[:], start=True, stop=True)
            adj_sb = const.tile([N, N], FP32)
            nc.scalar.activation(adj_sb[:], rowb_ps[:],
                                 mybir.ActivationFunctionType.Sigmoid,
                                 bias=col_sb[:])
            nc.vector.tensor_mul(adj_sb[:], adj_sb[:], mask_sb[:])
            nc.sync.dma_start(adj[:], adj_sb[:])
    nc.compile()
    return nc


def _build_launch2():
    nc = bacc.Bacc("TRN2", target_bir_lowering=False, debug=False,
                   num_devices=NCORES)
    NDC = HD // P  # 10 head-dim chunks
    wt2 = nc.dram_tensor("wt2", [FS, E], FP32, kind="ExternalInput")
    qTs = nc.dram_tensor("qTs", [P, NDC, N], FP32, kind="ExternalInput")
    kT = nc.dram_tensor("kT", [P, NDC, N], FP32, kind="ExternalInput")
    v = nc.dram_tensor("v", [N, FS], FP32, kind="ExternalInput")
    outp = nc.dram_tensor("outp", [N, E], FP32, kind="ExternalOutput")

    ET = 512  # out free-dim tile
    with tile.TileContext(nc) as tc:
        with (
            tc.tile_pool(name="const", bufs=1) as const,
            tc.tile_pool(name="w2", bufs=1) as w2p,
            tc.tile_pool(name="att_ps", bufs=1, space="PSUM") as attps,
            tc.tile_pool(name="ot_ps", bufs=2, space="PSUM") as otps,
            tc.tile_pool(name="out_ps", bufs=2, space="PSUM") as outps,
        ):
            # Prefetch the big weight slice first so DMA runs under attention.
            w2_sb = []
            for fc in range(FS // P):
                t = w2p.tile([P, E], FP32, tag=f"w2_{fc}", name=f"w2sb{fc}")
                nc.sync.dma_start(t[:], wt2[fc * P:(fc + 1) * P, :])
                w2_sb.append(t)

            qTs_sb = const.tile([P, NDC, N], FP32)
            nc.sync.dma_start(qTs_sb[:], qTs[:])
            kT_sb = const.tile([P, NDC, N], FP32)
            nc.sync.dma_start(kT_sb[:], kT[:])
            v_sb = const.tile([N, FS], FP32)
            nc.sync.dma_start(v_sb[:], v[:])

            ident = const.tile([N, N], FP32)
            make_identity(nc, ident[:])

            # scores[q, t] = sum_d qTs[d,q] kT[d,t]  (q pre-scaled by 1/sqrt(hd))
            sc_ps = attps.tile([N, N], FP32, tag="sc")
            for dc in range(NDC):
                nc.tensor.matmul(sc_ps[:], qTs_sb[:, dc, :], kT_sb[:, dc, :],
                                 start=(dc == 0), stop=(dc == NDC - 1))
            sc_sb = const.tile([N, N], FP32)
            nc.vector.tensor_copy(sc_sb[:], sc_ps[:])
            nmax = const.tile([N, 1], FP32)
            nc.vector.reduce_max(nmax[:], sc_sb[:], axis=mybir.AxisListType.X)
            nc.scalar.mul(nmax[:], nmax[:], -1.0)
            exp_sb = const.tile([N, N], FP32)
            nc.scalar.activation(exp_sb[:], sc_sb[:],
                                 mybir.ActivationFunctionType.Exp, bias=nmax[:])
            ssum = const.tile([N, 1], FP32)
            nc.vector.reduce_sum(ssum[:], exp_sb[:], axis=mybir.AxisListType.X)
            rec = const.tile([N, 1], FP32)
            nc.vector.reciprocal(rec[:], ssum[:])
            attn_sb = const.tile([N, N], FP32)
            nc.vector.tensor_scalar_mul(attn_sb[:], exp_sb[:], rec[:])

            # attn^T via PE transpose, then o^T[d, q] = sum_t v[t,d] attn^T[t,q]
            at_ps = attps.tile([N, N], FP32, tag="at")
            nc.tensor.transpose(at_ps[:], attn_sb[:], ident[:])
            attnT_sb = const.tile([N, N], FP32)
            nc.vector.tensor_copy(attnT_sb[:], at_ps[:])

            oT_sb = const.tile([P, FS // P, N], FP32)
            for b in range(FS // P):
                o_ps = otps.tile([P, N], FP32, tag="ot")
                nc.tensor.matmul(o_ps[:], v_sb[:, b * P:(b + 1) * P], attnT_sb[:],
                                 start=True, stop=True)
                nc.vector.tensor_copy(oT_sb[:, b, :], o_ps[:])

            # outp[n, e] = sum_f oT[f,n] * WoutT[f,e]  (partial over f slice)
            out_sb = const.tile([N, E], FP32)
            for ec in range(E // ET):
                op_ps = outps.tile([N, ET], FP32, tag="out")
                for fc in range(FS // P):
                    nc.tensor.matmul(
                        op_ps[:], oT_sb[:, fc, :],
                        w2_sb[fc][:, ec * ET:(ec + 1) * ET],
                        start=(fc == 0), stop=(fc == FS // P - 1),
                    )
                nc.vector.tensor_copy(out_sb[:, ec * ET:(ec + 1) * ET], op_ps[:])
            nc.sync.dma_start(outp[:], out_sb[:])
    nc.compile()
    return nc


def _chunked_T(a2d):
    """[R, C] -> transposed chunk layout [128, R//128, C] (k on partitions)."""
    r, c = a2d.shape
    return np.ascontiguousarray(
        a2d.reshape(r // P, P, c).transpose(1, 0, 2)).astype(np.float32)


def kernel(node_states, hist_prev, granger_w, granger_b,
           in_proj_w, in_proj_b, out_proj_w, out_proj_b, light_hertz):
    global LAST_RESULTS
    LAST_RESULTS = []
    node_states = np.asarray(node_states, dtype=np.float32)
    hist_prev = np.asarray(hist_prev, dtype=np.float32)
    granger_w = np.asarray(granger_w, dtype=np.float32)
    granger_b = np.asarray(granger_b, dtype=np.float32)
    in_proj_w = np.asarray(in_proj_w, dtype=np.float32)
    in_proj_b = np.asarray(in_proj_b, dtype=np.float32)
    out_proj_w = np.asarray(out_proj_w, dtype=np.float32)
    out_proj_b = np.asarray(out_proj_b, dtype=np.float32)

    x = node_states.reshape(N, E)
    hist = hist_prev.reshape(N, E)

    xT = _chunked_T(x.T.reshape(E, N))          # [128, 40, 27]
    histT = _chunked_T(hist.T.reshape(E, N))
    wh = _chunked_T(granger_w[0, :E].reshape(E, 1))
    wn = _chunked_T(granger_w[0, E:].reshape(E, 1))
    gb = granger_b.reshape(1, 1)
    mask = (1.0 - np.eye(N, dtype=np.float32))

    if "l1" not in _NC_CACHE:
        _NC_CACHE["l1"] = _build_launch1()
    if "l2" not in _NC_CACHE:
        _NC_CACHE["l2"] = _build_launch2()

    in_maps1 = []
    for c in range(NCORES):
        sl = slice(c * JC, (c + 1) * JC)
        in_maps1.append({
            "wt": np.ascontiguousarray(in_proj_w[sl, :].T),
            "xT": xT, "histT": histT, "wn": wn, "wh": wh, "gb": gb,
            "bias": in_proj_b[sl].reshape(1, JC),
            "mask": mask,
        })
    res1 = run_bass_kernel_spmd(_NC_CACHE["l1"], in_maps1, list(range(NCORES)))
    LAST_RESULTS.append(res1)

    qkv = np.concatenate([res1.results[c]["qkv"] for c in range(NCORES)], axis=1)
    causal_adj = np.asarray(res1.results[0]["adj"])

    q = qkv[:, :E]
    k = qkv[:, E:2 * E]
    v = qkv[:, 2 * E:]
    hw = float(np.clip(np.float32(light_hertz) / np.float32(1000.0), 0.1, 1.0))
    qscale = 1.0 / np.sqrt(np.float32(HD))

    in_maps2 = []
    for c in range(NCORES):
        h, half = divmod(c, 2)
        hsl = slice(h * HD, (h + 1) * HD)
        fsl = slice(h * HD + half * FS, h * HD + half * FS + FS)
        in_maps2.append({
            "wt2": np.ascontiguousarray(out_proj_w[:, fsl].T),
            "qTs": _chunked_T(np.ascontiguousarray(q[:, hsl].T) * qscale),
            "kT": _chunked_T(np.ascontiguousarray(k[:, hsl].T)),
            "v": np.ascontiguousarray(v[:, fsl]) * np.float32(hw),
        })
    res2 = run_bass_kernel_spmd(_NC_CACHE["l2"], in_maps2, list(range(NCORES)))
    LAST_RESULTS.append(res2)

    out = np.zeros((N, E), dtype=np.float32)
    for c in range(NCORES):
        out += res2.results[c]["outp"]
    out += np.float32(hw) * out_proj_b
    return out.reshape(N, D, 5), causal_adj


# revision 18
# speedup vs baseline: 1.4408x; 1.4408x over previous
"""Trainium2 Bass kernel for nn_CausalDiscoveryLayer (27-node Granger + MHA).

Contract: kernel(**inputs) takes FULL unsharded numpy inputs and returns the
full output (out [27,1024,5], causal_adj [27,27]) matching the reference.

Strategy (8 NeuronCores, tensor-parallel on the two big weight matrices):
  Launch 1: in_proj (15360x5120, 315MB) sharded 1920 rows/core; each core
            computes its qkv slice [27,1920]; Granger adjacency replicated.
  Host:     reassemble qkv, carve per-head q^T,k^T and v slices.
  Launch 2: out_proj (5120x5120, 105MB) sharded over the contraction dim
            (640/core = half a head); each core runs softmax-attention for
            its head and produces a partial output [27,5120]; host sums.

Weights are transposed on the host so that the contraction dimension lies on
SBUF partitions with unit-stride DMA (fp32 has no HW DMA-transpose path).
"""

import numpy as np

import concourse.bass as bass
import concourse.bacc as bacc
import concourse.mybir as mybir
import concourse.tile as tile
from concourse.bass_utils import run_bass_kernel_spmd
from concourse.masks import make_identity

N = 27
D = 1024
E = 5120
H = 4
HD = E // H          # 1280
NCORES = 8
P = 128
KC = E // P          # 40 contraction chunks of 128
JC = 3 * E // NCORES  # 1920 in_proj output cols per core
JT = 480             # matmul free-dim tile for launch 1 (4 * 480 = 1920)
FS = E // NCORES     # 640 out_proj contraction rows per core
FP32 = mybir.dt.float32
FP32R = mybir.dt.float32r  # single-pass PE mode (4x faster moving stream)
import os
MM_DT = FP32R if os.environ.get("KMM_DT", "f32r") == "f32r" else FP32

# Results of the last run (BassKernelResults per launch) for test harnesses.
LAST_RESULTS = []

_NC_CACHE = {}


def _build_launch1():
    nc = bacc.Bacc("TRN2", target_bir_lowering=False, debug=False,
                   num_devices=NCORES)
    wt = nc.dram_tensor("wt", [E, JC], MM_DT, kind="ExternalInput")
    xT = nc.dram_tensor("xT", [P, KC, N], FP32, kind="ExternalInput")
    xTr = nc.dram_tensor("xTr", [P, KC, N], MM_DT, kind="ExternalInput")
    histT = nc.dram_tensor("histT", [P, KC, N], FP32, kind="ExternalInput")
    wn = nc.dram_tensor("wn", [P, KC, 1], FP32, kind="ExternalInput")
    wh = nc.dram_tensor("wh", [P, KC, 1], FP32, kind="ExternalInput")
    gb = nc.dram_tensor("gb", [1, 1], FP32, kind="ExternalInput")
    bias = nc.dram_tensor("bias", [1, JC], MM_DT, kind="ExternalInput")
    mask = nc.dram_tensor("mask", [N, N], FP32, kind="ExternalInput")
    qkv = nc.dram_tensor("qkv", [N, JC], FP32, kind="ExternalOutput")
    adj = nc.dram_tensor("adj", [N, N], FP32, kind="ExternalOutput")

    with tile.TileContext(nc) as tc:
        with (
            tc.tile_pool(name="const", bufs=1) as const,
            tc.tile_pool(name="rhs", bufs=4) as rhsp,
            tc.tile_pool(name="outsb", bufs=1) as outsb,
            tc.tile_pool(name="acc", bufs=4, space="PSUM") as accp,
            tc.tile_pool(name="gps", bufs=1, space="PSUM") as gpsp,
        ):
            ones_f32 = const.tile([1, N], FP32)
            nc.gpsimd.memset(ones_f32[:], 1.0)
            ones = const.tile([1, N], MM_DT)
            nc.vector.tensor_copy(ones[:], ones_f32[:])
            xT_sb = const.tile([P, KC, N], FP32)
            nc.sync.dma_start(xT_sb[:], xT[:])
            xTr_sb = const.tile([P, KC, N], MM_DT)
            nc.sync.dma_start(xTr_sb[:], xTr[:])
            histT_sb = const.tile([P, KC, N], FP32)
            nc.sync.dma_start(histT_sb[:], histT[:])
            wn_sb = const.tile([P, KC, 1], FP32)
            nc.sync.dma_start(wn_sb[:], wn[:])
            wh_sb = const.tile([P, KC, 1], FP32)
            nc.sync.dma_start(wh_sb[:], wh[:])
            gb_sb = const.tile([1, 1], FP32)
            nc.sync.dma_start(gb_sb[:], gb[:])
            bias_sb = const.tile([1, JC], MM_DT)
            nc.sync.dma_start(bias_sb[:], bias[:])
            mask_sb = const.tile([N, N], FP32)
            nc.sync.dma_start(mask_sb[:], mask[:])

            out_sb = outsb.tile([N, JC], FP32)

            # Main projection: qkv_slice[n, j] = sum_k x[n,k] * W^T[k, j] + b[j]
            psums = [accp.tile([N, JT], FP32, tag="acc", name=f"acc{j}")
                     for j in range(JC // JT)]
            for kc in range(KC):
                rhs = rhsp.tile([P, JC], MM_DT)
                nc.sync.dma_start(rhs[:], wt[kc * P:(kc + 1) * P, :])
                for jc in range(JC // JT):
                    nc.tensor.matmul(
                        psums[jc][:],
                        xTr_sb[:, kc, :],
                        rhs[:, jc * JT:(jc + 1) * JT],
                        start=(kc == 0), stop=False,
                    )
            for jc in range(JC // JT):
                # bias broadcast: ones^T [N,1] @ bias_chunk [1,JT]
                nc.tensor.matmul(
                    psums[jc][:], ones[:],
                    bias_sb[:, jc * JT:(jc + 1) * JT],
                    start=False, stop=True,
                )
                nc.vector.tensor_copy(out_sb[:, jc * JT:(jc + 1) * JT], psums[jc][:])
            nc.sync.dma_start(qkv[:], out_sb[:])

            # Granger: col[i] = x[i,:].wn + gb ; row[j] = hist[j,:].wh
            col_ps = gpsp.tile([N, 1], FP32, tag="col")
            for kc in range(KC):
                nc.tensor.matmul(col_ps[:], xT_sb[:, kc, :], wn_sb[:, kc, :],
                                 start=(kc == 0), stop=False)
            nc.tensor.matmul(col_ps[:], ones_f32[:], gb_sb[:], start=False, stop=True)
            row_ps = gpsp.tile([1, N], FP32, tag="row")
            for kc in range(KC):
                nc.tensor.matmul(row_ps[:], wh_sb[:, kc, :], histT_sb[:, kc, :],
                                 start=(kc == 0), stop=(kc == KC - 1))
            col_sb = const.tile([N, 1], FP32)
            nc.vector.tensor_copy(col_sb[:], col_ps[:])
            row_sb = const.tile([1, N], FP32)
            nc.vector.tensor_copy(row_sb[:], row_ps[:])
            rowb_ps = gpsp.tile([N, N], FP32, tag="rowb")
            nc.tensor.matmul(rowb_ps[:], ones_f32[:], row_sb[:], start=True, stop=True)
            adj_sb = const.tile([N, N], FP32)
            nc.scalar.activation(adj_sb[:], rowb_ps[:],
                                 mybir.ActivationFunctionType.Sigmoid,
                                 bias=col_sb[:])
            nc.vector.tensor_mul(adj_sb[:], adj_sb[:], mask_sb[:])
            nc.sync.dma_start(adj[:], adj_sb[:])
    nc.compile()
    return nc


def _build_launch2():
    nc = bacc.Bacc("TRN2", target_bir_lowering=False, debug=False,
                   num_devices=NCORES)
    NDC = HD // P  # 10 head-dim chunks
    wt2 = nc.dram_tensor("wt2", [FS, E], MM_DT, kind="ExternalInput")
    qTs = nc.dram_tensor("qTs", [P, NDC, N], FP32, kind="ExternalInput")
    kT = nc.dram_tensor("kT", [P, NDC, N], FP32, kind="ExternalInput")
    v = nc.dram_tensor("v", [N, FS], FP32, kind="ExternalInput")
    outp = nc.dram_tensor("outp", [N, E], FP32, kind="ExternalOutput")

    ET = 512  # out free-dim tile
    with tile.TileContext(nc) as tc:
        with (
            tc.tile_pool(name="const", bufs=1) as const,
            tc.tile_pool(name="w2", bufs=1) as w2p,
            tc.tile_pool(name="att_ps", bufs=1, space="PSUM") as attps,
            tc.tile_pool(name="ot_ps", bufs=2, space="PSUM") as otps,
            tc.tile_pool(name="out_ps", bufs=2, space="PSUM") as outps,
        ):
            # Small attention inputs first (HWDGE FIFO), then the weight
            # slice in e-quarters fc-inner so the out loop can start early.
            qTs_sb = const.tile([P, NDC, N], FP32)
            nc.sync.dma_start(qTs_sb[:], qTs[:])
            kT_sb = const.tile([P, NDC, N], FP32)
            nc.sync.dma_start(kT_sb[:], kT[:])
            v_sb = const.tile([N, FS], FP32)
            nc.sync.dma_start(v_sb[:], v[:])

            EQ = E // 4
            w2_sb = []
            for fc in range(FS // P):
                t = w2p.tile([P, E], MM_DT, tag=f"w2_{fc}", name=f"w2sb{fc}")
                w2_sb.append(t)
            for eq in range(4):
                for fc in range(FS // P):
                    nc.sync.dma_start(
                        w2_sb[fc][:, eq * EQ:(eq + 1) * EQ],
                        wt2[fc * P:(fc + 1) * P, eq * EQ:(eq + 1) * EQ])

            ident = const.tile([N, N], FP32)
            make_identity(nc, ident[:])

            # scores[q, t] = sum_d qTs[d,q] kT[d,t]  (q pre-scaled by 1/sqrt(hd))
            sc_ps = attps.tile([N, N], FP32, tag="sc")
            for dc in range(NDC):
                nc.tensor.matmul(sc_ps[:], qTs_sb[:, dc, :], kT_sb[:, dc, :],
                                 start=(dc == 0), stop=(dc == NDC - 1))
            sc_sb = const.tile([N, N], FP32)
            nc.vector.tensor_copy(sc_sb[:], sc_ps[:])
            nmax = const.tile([N, 1], FP32)
            nc.vector.reduce_max(nmax[:], sc_sb[:], axis=mybir.AxisListType.X)
            nc.scalar.mul(nmax[:], nmax[:], -1.0)
            exp_sb = const.tile([N, N], FP32)
            nc.scalar.activation(exp_sb[:], sc_sb[:],
                                 mybir.ActivationFunctionType.Exp, bias=nmax[:])
            ssum = const.tile([N, 1], FP32)
            nc.vector.reduce_sum(ssum[:], exp_sb[:], axis=mybir.AxisListType.X)
            rec = const.tile([N, 1], FP32)
            nc.vector.reciprocal(rec[:], ssum[:])
            attn_sb = const.tile([N, N], FP32)
            nc.vector.tensor_scalar_mul(attn_sb[:], exp_sb[:], rec[:])

            # attn^T via PE transpose, then o^T[d, q] = sum_t v[t,d] attn^T[t,q]
            at_ps = attps.tile([N, N], FP32, tag="at")
            nc.tensor.transpose(at_ps[:], attn_sb[:], ident[:])
            attnT_sb = const.tile([N, N], FP32)
            nc.vector.tensor_copy(attnT_sb[:], at_ps[:])

            oT_sb = const.tile([P, FS // P, N], MM_DT)
            for b in range(FS // P):
                o_ps = otps.tile([P, N], FP32, tag="ot")
                nc.tensor.matmul(o_ps[:], v_sb[:, b * P:(b + 1) * P], attnT_sb[:],
                                 start=True, stop=True)
                nc.vector.tensor_copy(oT_sb[:, b, :], o_ps[:])

            # outp[n, e] = sum_f oT[f,n] * WoutT[f,e]  (partial over f slice)
            out_sb = const.tile([N, E], FP32)
            for ec in range(E // ET):
                op_ps = outps.tile([N, ET], FP32, tag="out")
                for fc in range(FS // P):
                    nc.tensor.matmul(
                        op_ps[:], oT_sb[:, fc, :],
                        w2_sb[fc][:, ec * ET:(ec + 1) * ET],
                        start=(fc == 0), stop=(fc == FS // P - 1),
                    )
                nc.vector.tensor_copy(out_sb[:, ec * ET:(ec + 1) * ET], op_ps[:])
            nc.sync.dma_start(outp[:], out_sb[:])
    nc.compile()
    return nc


def _chunked_T(a2d):
    """[R, C] -> transposed chunk layout [128, R//128, C] (k on partitions)."""
    r, c = a2d.shape
    return np.ascontiguousarray(
        a2d.reshape(r // P, P, c).transpose(1, 0, 2)).astype(np.float32)


def kernel(node_states, hist_prev, granger_w, granger_b,
           in_proj_w, in_proj_b, out_proj_w, out_proj_b, light_hertz):
    global LAST_RESULTS
    LAST_RESULTS = []
    node_states = np.asarray(node_states, dtype=np.float32)
    hist_prev = np.asarray(hist_prev, dtype=np.float32)
    granger_w = np.asarray(granger_w, dtype=np.float32)
    granger_b = np.asarray(granger_b, dtype=np.float32)
    in_proj_w = np.asarray(in_proj_w, dtype=np.float32)
    in_proj_b = np.asarray(in_proj_b, dtype=np.float32)
    out_proj_w = np.asarray(out_proj_w, dtype=np.float32)
    out_proj_b = np.asarray(out_proj_b, dtype=np.float32)

    x = node_states.reshape(N, E)
    hist = hist_prev.reshape(N, E)

    xT = _chunked_T(x.T.reshape(E, N))          # [128, 40, 27]
    histT = _chunked_T(hist.T.reshape(E, N))
    wh = _chunked_T(granger_w[0, :E].reshape(E, 1))
    wn = _chunked_T(granger_w[0, E:].reshape(E, 1))
    gb = granger_b.reshape(1, 1)
    mask = (1.0 - np.eye(N, dtype=np.float32))

    if "l1" not in _NC_CACHE:
        _NC_CACHE["l1"] = _build_launch1()
    if "l2" not in _NC_CACHE:
        _NC_CACHE["l2"] = _build_launch2()

    in_maps1 = []
    for c in range(NCORES):
        sl = slice(c * JC, (c + 1) * JC)
        in_maps1.append({
            "wt": np.ascontiguousarray(in_proj_w[sl, :].T),
            "xT": xT, "xTr": xT, "histT": histT, "wn": wn, "wh": wh, "gb": gb,
            "bias": in_proj_b[sl].reshape(1, JC),
            "mask": mask,
        })
    res1 = run_bass_kernel_spmd(_NC_CACHE["l1"], in_maps1, list(range(NCORES)))
    LAST_RESULTS.append(res1)

    qkv = np.concatenate([res1.results[c]["qkv"] for c in range(NCORES)], axis=1)
    causal_adj = np.asarray(res1.results[0]["adj"])

    q = qkv[:, :E]
    k = qkv[:, E:2 * E]
    v = qkv[:, 2 * E:]
    hw = float(np.clip(np.float32(light_hertz) / np.float32(1000.0), 0.1, 1.0))
    qscale = 1.0 / np.sqrt(np.float32(HD))

    in_maps2 = []
    for c in range(NCORES):
        h, half = divmod(c, 2)
        hsl = slice(h * HD, (h + 1) * HD)
        fsl = slice(h * HD + half * FS, h * HD + half * FS + FS)
        in_maps2.append({
            "wt2": np.ascontiguousarray(out_proj_w[:, fsl].T),
            "qTs": _chunked_T(np.ascontiguousarray(q[:, hsl].T) * qscale),
            "kT": _chunked_T(np.ascontiguousarray(k[:, hsl].T)),
            "v": np.ascontiguousarray(v[:, fsl]) * np.float32(hw),
        })
    res2 = run_bass_kernel_spmd(_NC_CACHE["l2"], in_maps2, list(range(NCORES)))
    LAST_RESULTS.append(res2)

    out = np.zeros((N, E), dtype=np.float32)
    for c in range(NCORES):
        out += res2.results[c]["outp"]
    out += np.float32(hw) * out_proj_b
    return out.reshape(N, D, 5), causal_adj
